# revision 9
# baseline (speedup 1.0000x reference)
"""Luong attention Trainium2 kernel, data-parallel over batch on 8 NeuronCores.

Per core (one batch element b):
    keys^T[e,k] = sum_d Wa[d,e] * enc[k,d]          (bias ba dropped: it only
                                                     shifts scores by a per-row
                                                     constant, which softmax is
                                                     invariant to -- exact)
    S[q,k]     = sum_e dec[q,e] * keysT[e,k]
    U          = exp(S - rowmax(S)),  r = 1/rowsum(U)
    alignment  = U * r                               (f32 output)
    context    = (U @ enc) * r                       (f32 output)

All matmuls run in fp16 (inputs cast during SWDGE DMA), accumulation in f32
PSUM. Transposes (dec, enc, U) run on the tensor engine (is_transpose matmul
against an identity), 8 tiles packed per PSUM bank, evicted with one wide
copy. Phase C is software-pipelined two q-blocks deep so the PE never stalls
on softmax/transpose latency.
"""

import numpy as np

B, TQ, TK, D = 8, 2048, 2048, 1024
N_CORES = 8
P = 128
KO = TK // P  # 16  k-blocks
QO = TQ // P  # 16  q-blocks
DO = D // P   # 8   feature blocks
FD = 512      # matmul moving free dim
NK = TK // FD  # 4  score psum banks
NE = D // FD   # 2  context psum banks


def _emit(nc, dec_h, enc_h, wa_h, ctx_h, aln_h):
    import concourse.mybir as mybir
    import concourse.tile as tile
    from concourse.masks import make_identity

    f32 = mybir.dt.float32
    f16 = mybir.dt.float16
    AF = mybir.ActivationFunctionType
    AX = mybir.AxisListType

    dec3 = dec_h[:].rearrange("(qo qp) d -> qp qo d", qp=P)
    enc3 = enc_h[:].rearrange("(ko kp) d -> kp ko d", kp=P)
    wa3 = wa_h[:].rearrange("(do dp) e -> dp do e", dp=P)
    ctx3 = ctx_h[:].rearrange("(qo qp) e -> qp qo e", qp=P)
    aln3 = aln_h[:].rearrange("(qo qp) k -> qp qo k", qp=P)

    with tile.TileContext(nc) as tc:
        with (
            tc.tile_pool(name="const", bufs=1) as const,
            tc.tile_pool(name="persist", bufs=1) as persist,
            tc.tile_pool(name="work", bufs=2) as work,
            tc.tile_pool(name="stats", bufs=4) as stats,
        ):
            ident = const.tile([P, P], f16)
            make_identity(nc, ident[:])

            # tensors live for the whole kernel
            # decT[dp, qo, do, j] = dec[qo*P+j, do*P+dp]
            enc_bf = persist.tile([P, KO, D], f16)     # enc, natural [k,d]
            decT = persist.tile([P, QO, DO, P], f16)   # dec^T  [e, q]
            keysT = persist.tile([P, DO, TK], f16)     # keys^T [e, k]

            # phase A/B staging (stage pool is LIFO-innermost, closed before
            # phase C so its SBUF is never reused underneath live tiles)
            with (
                tc.tile_pool(name="stage", bufs=1) as stage,
                tc.tile_pool(name="psumA", bufs=2, space="PSUM") as psumA,
            ):
                wa_bf = stage.tile([P, DO, D], f16)
                nc.gpsimd.dma_start(out=wa_bf[:], in_=wa3)

                # dec: chunked cast-load, PE-transpose each chunk into decT
                for qo in range(QO):
                    dchunk = stage.tile([P, D], f16, tag="dchunk", bufs=3)
                    nc.gpsimd.dma_start(out=dchunk[:], in_=dec3[:, qo, :])
                    tr = psumA.tile([P, DO, P], f16, tag="trps")
                    for do in range(DO):
                        nc.tensor.transpose(
                            tr[:, do, :], dchunk[:, do * P:(do + 1) * P], ident[:]
                        )
                    if qo % 2 == 0:
                        nc.scalar.copy(decT[:, qo, :, :], tr[:])
                    else:
                        nc.vector.tensor_copy(decT[:, qo, :, :], tr[:])

                # enc: cast-load resident, PE-transpose into encT
                # encT[dp, ko, do, j] = enc[ko*P+j, do*P+dp]
                for c in range(4):
                    nc.gpsimd.dma_start(
                        out=enc_bf[:, c * 4:(c + 1) * 4, :],
                        in_=enc3[:, c * 4:(c + 1) * 4, :],
                    )
                encT = stage.tile([P, KO, DO, P], f16)
                for ko in range(KO):
                    tr = psumA.tile([P, DO, P], f16, tag="trps")
                    for do in range(DO):
                        nc.tensor.transpose(
                            tr[:, do, :], enc_bf[:, ko, do * P:(do + 1) * P],
                            ident[:],
                        )
                    if ko % 2 == 0:
                        nc.scalar.copy(encT[:, ko, :, :], tr[:])
                    else:
                        nc.vector.tensor_copy(encT[:, ko, :, :], tr[:])

                # phase B: keysT[e,k] = Wa^T @ enc^T (kc-outer so scores can
                # start on k-chunk 0 early)
                with tc.tile_pool(name="psumB", bufs=2, space="PSUM") as psumB:
                    for kc in range(NK):
                        for eo in range(DO):
                            pt = psumB.tile([P, FD], f32, tag="ptB")
                            for do in range(DO):
                                nc.tensor.matmul(
                                    pt[:],
                                    wa_bf[:, do, eo * P:(eo + 1) * P],
                                    encT[:, kc * 4:(kc + 1) * 4, do, :],
                                    start=(do == 0),
                                    stop=(do == DO - 1),
                                )
                            nc.scalar.copy(
                                keysT[:, eo, kc * FD:(kc + 1) * FD], pt[:]
                            )

            # phase C: per q-block scores -> softmax -> (transpose U) -> context
            with (
                tc.tile_pool(name="spsum", bufs=1, space="PSUM") as spsum,
                tc.tile_pool(name="upsum", bufs=1, space="PSUM") as upsum,
                tc.tile_pool(name="cpsum", bufs=1, space="PSUM") as cpsum,
            ):
                u_t = [None] * QO
                uT_t = [None] * QO
                r_t = [None] * QO

                def scores_softmax(qb):
                    s_ps = spsum.tile([P, NK, FD], f32, tag="s")
                    for kc in range(NK):
                        for eo in range(DO):
                            nc.tensor.matmul(
                                s_ps[:, kc, :],
                                decT[:, qb, eo, :],
                                keysT[:, eo, kc * FD:(kc + 1) * FD],
                                start=(eo == 0),
                                stop=(eo == DO - 1),
                            )
                    neg_max = stats.tile([P, 1], f32, tag="m")
                    nc.vector.tensor_reduce(
                        neg_max[:], s_ps[:], axis=AX.XY,
                        op=mybir.AluOpType.max, negate=True,
                    )
                    u = work.tile([P, TK], f16, tag="u")
                    sumexp = stats.tile([P, 1], f32, tag="se")
                    nc.scalar.activation(
                        u[:], s_ps[:].rearrange("p a b -> p (a b)"), AF.Exp,
                        bias=neg_max[:], scale=1.0, accum_out=sumexp[:],
                    )
                    recip = stats.tile([P, 1], f32, tag="r", bufs=4)
                    nc.vector.reciprocal(recip[:], sumexp[:])
                    a32 = work.tile([P, TK], f32, tag="a32")
                    nc.vector.tensor_scalar_mul(a32[:], u[:], recip[:])
                    nc.sync.dma_start(aln3[:, qb, :], a32[:])
                    u_t[qb], r_t[qb] = u, recip

                def transpose_u(qb):
                    u = u_t[qb]
                    ut_ps = upsum.tile([P, KO, P], f16, tag="ut")
                    for kt in range(KO):
                        nc.tensor.transpose(
                            ut_ps[:, kt, :], u[:, kt * P:(kt + 1) * P], ident[:]
                        )
                    uT = work.tile([P, KO, P], f16, tag="uT")
                    nc.scalar.copy(uT[:], ut_ps[:])
                    uT_t[qb] = uT

                def context(qb):
                    uT = uT_t[qb]
                    c_ps = cpsum.tile([P, NE, FD], f32, tag="c")
                    for kt in range(KO):
                        for ec in range(NE):
                            nc.tensor.matmul(
                                c_ps[:, ec, :],
                                uT[:, kt, :],
                                enc_bf[:, kt, ec * FD:(ec + 1) * FD],
                                start=(kt == 0),
                                stop=(kt == KO - 1),
                            )
                    c_sb = work.tile([P, D], f32, tag="c_sb")
                    nc.scalar.activation(
                        c_sb[:], c_ps[:].rearrange("p a b -> p (a b)"),
                        AF.Copy, scale=r_t[qb][:],
                    )
                    nc.sync.dma_start(ctx3[:, qb, :], c_sb[:])

                for qb in range(QO):
                    scores_softmax(qb)
                    if qb >= 1:
                        transpose_u(qb - 1)
                    if qb >= 2:
                        context(qb - 2)
                transpose_u(QO - 1)
                context(QO - 2)
                context(QO - 1)


def build_program():
    import concourse.bacc as bacc
    import concourse.mybir as mybir

    f32 = mybir.dt.float32
    # Bacc (not raw Bass): its compile() pass legalizes multi-wait
    # instructions (move_matmul_waits_to_ldweights, generate_event_semaphores)
    # -- walrus codegen only supports ONE embedded sync wait per instruction.
    nc = bacc.Bacc(None, target_bir_lowering=False)
    dec_h = nc.declare_dram_parameter("dec", [TQ, D], f32, isOutput=False)
    enc_h = nc.declare_dram_parameter("enc", [TK, D], f32, isOutput=False)
    wa_h = nc.declare_dram_parameter("wa", [D, D], f32, isOutput=False)
    ctx_h = nc.declare_dram_parameter("ctx_out", [TQ, D], f32, isOutput=True)
    aln_h = nc.declare_dram_parameter("aln_out", [TQ, TK], f32, isOutput=True)
    _emit(nc, dec_h, enc_h, wa_h, ctx_h, aln_h)
    nc.finalize()
    return nc


_LAST_RESULT = {}


def _jit_spmd(nc):
    """Mirror bass2jax.run_bass_via_pjrt's multi-core path, but return the
    jitted function + input-staging helpers so executions can be timed."""
    import concourse.mybir as mybir
    import jax
    from jax.sharding import Mesh, PartitionSpec
    from jax.experimental.shard_map import shard_map
    from concourse import bass2jax

    bass2jax.install_neuronx_cc_hook()

    partition_name = (
        nc.partition_id_tensor.name if nc.partition_id_tensor else None
    )
    in_names, out_names, out_avals, zero_outs = [], [], [], []
    for alloc in nc.m.functions[0].allocations:
        if not isinstance(alloc, mybir.MemoryLocationSet):
            continue
        name = alloc.memorylocations[0].name
        if alloc.kind == "ExternalInput":
            if name != partition_name:
                in_names.append(name)
        elif alloc.kind == "ExternalOutput":
            out_names.append(name)
            shape = tuple(alloc.tensor_shape)
            dtype = mybir.dt.np(alloc.dtype)
            out_avals.append(jax.core.ShapedArray(shape, dtype))
            zero_outs.append(np.zeros(shape, dtype))
    n_params = len(in_names)
    all_names = in_names + out_names
    if partition_name is not None:
        all_names = all_names + [partition_name]

    def _body(*args):
        operands = list(args)
        if partition_name is not None:
            operands.append(bass2jax.partition_id_tensor())
        outs = bass2jax._bass_exec_p.bind(
            *operands,
            out_avals=tuple(out_avals),
            in_names=tuple(all_names),
            out_names=tuple(out_names),
            lowering_input_output_aliases=(),
            sim_require_finite=True,
            sim_require_nnan=True,
            nc=nc,
        )
        return tuple(outs)

    devices = jax.devices()[:N_CORES]
    mesh = Mesh(np.asarray(devices), ("core",))
    in_specs = (PartitionSpec("core"),) * (n_params + len(out_names))
    out_specs = (PartitionSpec("core"),) * len(out_names)
    fn = jax.jit(
        shard_map(_body, mesh=mesh, in_specs=in_specs, out_specs=out_specs,
                  check_rep=False),
        keep_unused=True,
    )
    return fn, in_names, out_names, zero_outs, mesh


def _stage_inputs(in_maps, in_names, zero_outs, mesh):
    import jax
    from jax.sharding import NamedSharding, PartitionSpec

    sharding = NamedSharding(mesh, PartitionSpec("core"))
    args = []
    for name in in_names:
        cat = np.concatenate([np.asarray(m[name]) for m in in_maps], axis=0)
        args.append(jax.device_put(cat, sharding))
    for z in zero_outs:
        cat = np.concatenate([z] * N_CORES, axis=0)
        args.append(jax.device_put(cat, sharding))
    return args


def build_baseline_program():
    """Same I/O signature, near-zero work: used to subtract dispatch cost."""
    import concourse.bacc as bacc
    import concourse.mybir as mybir
    import concourse.tile as tile

    f32 = mybir.dt.float32
    nc = bacc.Bacc(None, target_bir_lowering=False)
    dec_h = nc.declare_dram_parameter("dec", [TQ, D], f32, isOutput=False)
    nc.declare_dram_parameter("enc", [TK, D], f32, isOutput=False)
    nc.declare_dram_parameter("wa", [D, D], f32, isOutput=False)
    ctx_h = nc.declare_dram_parameter("ctx_out", [TQ, D], f32, isOutput=True)
    aln_h = nc.declare_dram_parameter("aln_out", [TQ, TK], f32, isOutput=True)
    with tile.TileContext(nc) as tc:
        with tc.tile_pool(name="p", bufs=1) as pool:
            t = pool.tile([P, P], f32)
            nc.sync.dma_start(out=t[:], in_=dec_h[:P, :P])
            nc.sync.dma_start(out=ctx_h[:P, :P], in_=t[:])
            nc.sync.dma_start(out=aln_h[:P, :P], in_=t[:])
    nc.finalize()
    return nc


def time_kernel(inputs, reps=10):
    """Median wall time of the real NEFF minus the empty-NEFF baseline, ns."""
    import time as _time

    import jax

    dec = np.ascontiguousarray(np.asarray(inputs["decoder_output"], np.float32))
    enc = np.ascontiguousarray(np.asarray(inputs["encoder_output"], np.float32))
    wa = np.ascontiguousarray(np.asarray(inputs["Wa"], np.float32))
    in_maps = [{"dec": dec[b], "enc": enc[b], "wa": wa} for b in range(B)]

    def run(nc_builder):
        nc = nc_builder()
        fn, in_names, out_names, zero_outs, mesh = _jit_spmd(nc)
        args = _stage_inputs(in_maps, in_names, zero_outs, mesh)
        out = fn(*args)  # compile + first run
        jax.block_until_ready(out)
        times = []
        for _ in range(reps):
            t0 = _time.perf_counter()
            out = fn(*args)
            jax.block_until_ready(out)
            times.append(_time.perf_counter() - t0)
        return np.median(times), np.min(times), out

    t_base, t_base_min, _ = run(build_baseline_program)
    t_real, t_real_min, out = run(build_program)
    print(f"  baseline median={t_base*1e3:.3f}ms min={t_base_min*1e3:.3f}ms")
    print(f"  real     median={t_real*1e3:.3f}ms min={t_real_min*1e3:.3f}ms")
    return (t_real - t_base) * 1e9


def kernel(decoder_output, encoder_output, Wa, ba):
    # ba is mathematically irrelevant: it shifts each score row by a
    # per-q constant (dec[q,:] @ ba), and softmax is shift-invariant.
    from concourse.bass_utils import run_bass_kernel_spmd

    nc = build_program()
    dec = np.ascontiguousarray(np.asarray(decoder_output, dtype=np.float32))
    enc = np.ascontiguousarray(np.asarray(encoder_output, dtype=np.float32))
    wa = np.ascontiguousarray(np.asarray(Wa, dtype=np.float32))
    in_maps = [
        {"dec": dec[b], "enc": enc[b], "wa": wa} for b in range(B)
    ]
    res = run_bass_kernel_spmd(nc, in_maps, core_ids=list(range(N_CORES)))
    _LAST_RESULT["res"] = res
    context = np.stack([res.results[b]["ctx_out"] for b in range(B)])
    alignment = np.stack([res.results[b]["aln_out"] for b in range(B)])
    return context, alignment


# revision 12
# speedup vs baseline: 106.3332x; 106.3332x over previous
"""Luong attention Trainium2 kernel, data-parallel over batch on 8 NeuronCores.

Per core (one batch element b):
    keys^T[e,k] = sum_d Wa[d,e] * enc[k,d]          (bias ba dropped: it only
                                                     shifts scores by a per-row
                                                     constant, which softmax is
                                                     invariant to -- exact)
    S[q,k]     = sum_e dec[q,e] * keysT[e,k]
    U          = exp(S - rowmax(S)),  r = 1/rowsum(U)
    alignment  = U * r                               (f32 output)
    context    = (U @ enc) * r                       (f32 output)

All matmuls run in fp16 (inputs cast during SWDGE DMA), accumulation in f32
PSUM. Transposes (dec, enc, U) run on the tensor engine (is_transpose matmul
against an identity), 8 tiles packed per PSUM bank, evicted with one wide
copy. Phase C is software-pipelined two q-blocks deep so the PE never stalls
on softmax/transpose latency.
"""

import numpy as np

B, TQ, TK, D = 8, 2048, 2048, 1024
N_CORES = 8
P = 128
KO = TK // P  # 16  k-blocks
QO = TQ // P  # 16  q-blocks
DO = D // P   # 8   feature blocks
FD = 512      # matmul moving free dim
NK = TK // FD  # 4  score psum banks
NE = D // FD   # 2  context psum banks


def _emit(nc, dec_h, enc_h, wa_h, ctx_h, aln_h):
    import concourse.tile as tile

    with tile.TileContext(nc) as tc:
        _emit_core(nc, tc, dec_h, enc_h, wa_h, ctx_h, aln_h)


def _emit_core(nc, tc, dec_h, enc_h, wa_h, ctx_h, aln_h):
    import concourse.mybir as mybir
    from concourse.masks import make_identity

    f32 = mybir.dt.float32
    f16 = mybir.dt.float16
    AF = mybir.ActivationFunctionType
    AX = mybir.AxisListType

    dec3 = dec_h[:].rearrange("(qo qp) d -> qp qo d", qp=P)
    enc3 = enc_h[:].rearrange("(ko kp) d -> kp ko d", kp=P)
    wa3 = wa_h[:].rearrange("(do dp) e -> dp do e", dp=P)
    ctx3 = ctx_h[:].rearrange("(qo qp) e -> qp qo e", qp=P)
    aln3 = aln_h[:].rearrange("(qo qp) k -> qp qo k", qp=P)

    if True:
        with (
            tc.tile_pool(name="const", bufs=1) as const,
            tc.tile_pool(name="persist", bufs=1) as persist,
            tc.tile_pool(name="work", bufs=2) as work,
            tc.tile_pool(name="stats", bufs=4) as stats,
        ):
            ident = const.tile([P, P], f16)
            make_identity(nc, ident[:])

            # tensors live for the whole kernel
            # decT[dp, qo, do, j] = dec[qo*P+j, do*P+dp]
            enc_bf = persist.tile([P, KO, D], f16)     # enc, natural [k,d]
            decT = persist.tile([P, QO, DO, P], f16)   # dec^T  [e, q]
            keysT = persist.tile([P, DO, TK], f16)     # keys^T [e, k]

            # phase A/B staging (stage pool is LIFO-innermost, closed before
            # phase C so its SBUF is never reused underneath live tiles)
            with (
                tc.tile_pool(name="stage", bufs=1) as stage,
                tc.tile_pool(name="psumA", bufs=2, space="PSUM") as psumA,
            ):
                wa_bf = stage.tile([P, DO, D], f16)
                encT = stage.tile([P, KO, DO, P], f16)

                def load_dec(qo):
                    dchunk = stage.tile([P, D], f16, tag="dchunk", bufs=3)
                    nc.gpsimd.dma_start(out=dchunk[:], in_=dec3[:, qo, :])
                    tr = psumA.tile([P, DO, P], f16, tag="trps")
                    for do in range(DO):
                        nc.tensor.transpose(
                            tr[:, do, :], dchunk[:, do * P:(do + 1) * P], ident[:]
                        )
                    if qo % 2 == 0:
                        nc.scalar.copy(decT[:, qo, :, :], tr[:])
                    else:
                        nc.vector.tensor_copy(decT[:, qo, :, :], tr[:])

                def transpose_enc(ko):
                    # encT[dp, ko, do, j] = enc[ko*P+j, do*P+dp]
                    tr = psumA.tile([P, DO, P], f16, tag="trps")
                    for do in range(DO):
                        nc.tensor.transpose(
                            tr[:, do, :], enc_bf[:, ko, do * P:(do + 1) * P],
                            ident[:],
                        )
                    if ko % 2 == 0:
                        nc.scalar.copy(encT[:, ko, :, :], tr[:])
                    else:
                        nc.vector.tensor_copy(encT[:, ko, :, :], tr[:])

                # interleave loads so the PE starts transposing at ~2us and
                # enc chunks arrive while dec transposes run
                load_dec(0)
                load_dec(1)
                nc.gpsimd.dma_start(out=wa_bf[:], in_=wa3)
                for c in range(4):
                    nc.gpsimd.dma_start(
                        out=enc_bf[:, c * 4:(c + 1) * 4, :],
                        in_=enc3[:, c * 4:(c + 1) * 4, :],
                    )
                    for qo in range(2 + c * 3, min(QO, 5 + c * 3)):
                        load_dec(qo)
                for qo in range(14, QO):
                    load_dec(qo)
                for ko in range(KO):
                    transpose_enc(ko)

                # phase B: keysT[e,k] = Wa^T @ enc^T (kc-outer so scores can
                # start on k-chunk 0 early)
                with tc.tile_pool(name="psumB", bufs=2, space="PSUM") as psumB:
                    for kc in range(NK):
                        for eo in range(DO):
                            pt = psumB.tile([P, FD], f32, tag="ptB")
                            for do in range(DO):
                                nc.tensor.matmul(
                                    pt[:],
                                    wa_bf[:, do, eo * P:(eo + 1) * P],
                                    encT[:, kc * 4:(kc + 1) * 4, do, :],
                                    start=(do == 0),
                                    stop=(do == DO - 1),
                                )
                            nc.scalar.copy(
                                keysT[:, eo, kc * FD:(kc + 1) * FD], pt[:]
                            )

            # phase C: per q-block scores -> softmax -> (transpose U) -> context
            with (
                tc.tile_pool(name="spsum", bufs=1, space="PSUM") as spsum,
                tc.tile_pool(name="upsum", bufs=1, space="PSUM") as upsum,
                tc.tile_pool(name="cpsum", bufs=1, space="PSUM") as cpsum,
            ):
                u_t = [None] * QO
                uT_t = [None] * QO
                r_t = [None] * QO

                def scores_softmax(qb):
                    s_ps = spsum.tile([P, NK, FD], f32, tag="s")
                    for kc in range(NK):
                        for eo in range(DO):
                            nc.tensor.matmul(
                                s_ps[:, kc, :],
                                decT[:, qb, eo, :],
                                keysT[:, eo, kc * FD:(kc + 1) * FD],
                                start=(eo == 0),
                                stop=(eo == DO - 1),
                            )
                    neg_max = stats.tile([P, 1], f32, tag="m")
                    nc.vector.tensor_reduce(
                        neg_max[:], s_ps[:], axis=AX.XY,
                        op=mybir.AluOpType.max, negate=True,
                    )
                    u = work.tile([P, TK], f16, tag="u")
                    sumexp = stats.tile([P, 1], f32, tag="se")
                    nc.scalar.activation(
                        u[:], s_ps[:].rearrange("p a b -> p (a b)"), AF.Exp,
                        bias=neg_max[:], scale=1.0, accum_out=sumexp[:],
                    )
                    recip = stats.tile([P, 1], f32, tag="r", bufs=4)
                    nc.vector.reciprocal(recip[:], sumexp[:])
                    a32 = work.tile([P, TK], f32, tag="a32")
                    nc.vector.tensor_scalar_mul(a32[:], u[:], recip[:])
                    nc.sync.dma_start(aln3[:, qb, :], a32[:])
                    u_t[qb], r_t[qb] = u, recip

                def transpose_u(qb):
                    u = u_t[qb]
                    ut_ps = upsum.tile([P, KO, P], f16, tag="ut")
                    for kt in range(KO):
                        nc.tensor.transpose(
                            ut_ps[:, kt, :], u[:, kt * P:(kt + 1) * P], ident[:]
                        )
                    uT = work.tile([P, KO, P], f16, tag="uT")
                    nc.scalar.copy(uT[:], ut_ps[:])
                    uT_t[qb] = uT

                def context(qb):
                    uT = uT_t[qb]
                    c_ps = cpsum.tile([P, NE, FD], f32, tag="c")
                    for kt in range(KO):
                        for ec in range(NE):
                            nc.tensor.matmul(
                                c_ps[:, ec, :],
                                uT[:, kt, :],
                                enc_bf[:, kt, ec * FD:(ec + 1) * FD],
                                start=(kt == 0),
                                stop=(kt == KO - 1),
                            )
                    c_sb = work.tile([P, D], f32, tag="c_sb")
                    nc.scalar.activation(
                        c_sb[:], c_ps[:].rearrange("p a b -> p (a b)"),
                        AF.Copy, scale=r_t[qb][:],
                    )
                    nc.sync.dma_start(ctx3[:, qb, :], c_sb[:])

                for qb in range(QO):
                    scores_softmax(qb)
                    if qb >= 1:
                        transpose_u(qb - 1)
                    if qb >= 2:
                        context(qb - 2)
                transpose_u(QO - 1)
                context(QO - 2)
                context(QO - 1)


def build_program():
    import concourse.bacc as bacc
    import concourse.mybir as mybir

    f32 = mybir.dt.float32
    # Bacc (not raw Bass): its compile() pass legalizes multi-wait
    # instructions (move_matmul_waits_to_ldweights, generate_event_semaphores)
    # -- walrus codegen only supports ONE embedded sync wait per instruction.
    nc = bacc.Bacc(None, target_bir_lowering=False)
    dec_h = nc.declare_dram_parameter("dec", [TQ, D], f32, isOutput=False)
    enc_h = nc.declare_dram_parameter("enc", [TK, D], f32, isOutput=False)
    wa_h = nc.declare_dram_parameter("wa", [D, D], f32, isOutput=False)
    ctx_h = nc.declare_dram_parameter("ctx_out", [TQ, D], f32, isOutput=True)
    aln_h = nc.declare_dram_parameter("aln_out", [TQ, TK], f32, isOutput=True)
    _emit(nc, dec_h, enc_h, wa_h, ctx_h, aln_h)
    nc.finalize()
    return nc


_LAST_RESULT = {}


def _jit_spmd(nc):
    """Mirror bass2jax.run_bass_via_pjrt's multi-core path, but return the
    jitted function + input-staging helpers so executions can be timed."""
    import concourse.mybir as mybir
    import jax
    from jax.sharding import Mesh, PartitionSpec
    from jax.experimental.shard_map import shard_map
    from concourse import bass2jax

    bass2jax.install_neuronx_cc_hook()

    partition_name = (
        nc.partition_id_tensor.name if nc.partition_id_tensor else None
    )
    in_names, out_names, out_avals, zero_outs = [], [], [], []
    for alloc in nc.m.functions[0].allocations:
        if not isinstance(alloc, mybir.MemoryLocationSet):
            continue
        name = alloc.memorylocations[0].name
        if alloc.kind == "ExternalInput":
            if name != partition_name:
                in_names.append(name)
        elif alloc.kind == "ExternalOutput":
            out_names.append(name)
            shape = tuple(alloc.tensor_shape)
            dtype = mybir.dt.np(alloc.dtype)
            out_avals.append(jax.core.ShapedArray(shape, dtype))
            zero_outs.append(np.zeros(shape, dtype))
    n_params = len(in_names)
    all_names = in_names + out_names
    if partition_name is not None:
        all_names = all_names + [partition_name]

    def _body(*args):
        operands = list(args)
        if partition_name is not None:
            operands.append(bass2jax.partition_id_tensor())
        outs = bass2jax._bass_exec_p.bind(
            *operands,
            out_avals=tuple(out_avals),
            in_names=tuple(all_names),
            out_names=tuple(out_names),
            lowering_input_output_aliases=(),
            sim_require_finite=True,
            sim_require_nnan=True,
            nc=nc,
        )
        return tuple(outs)

    devices = jax.devices()[:N_CORES]
    mesh = Mesh(np.asarray(devices), ("core",))
    in_specs = (PartitionSpec("core"),) * (n_params + len(out_names))
    out_specs = (PartitionSpec("core"),) * len(out_names)
    fn = jax.jit(
        shard_map(_body, mesh=mesh, in_specs=in_specs, out_specs=out_specs,
                  check_rep=False),
        keep_unused=True,
    )
    return fn, in_names, out_names, zero_outs, mesh


def _stage_inputs(in_maps, in_names, zero_outs, mesh):
    import jax
    from jax.sharding import NamedSharding, PartitionSpec

    sharding = NamedSharding(mesh, PartitionSpec("core"))
    args = []
    for name in in_names:
        cat = np.concatenate([np.asarray(m[name]) for m in in_maps], axis=0)
        args.append(jax.device_put(cat, sharding))
    for z in zero_outs:
        cat = np.concatenate([z] * N_CORES, axis=0)
        args.append(jax.device_put(cat, sharding))
    return args


def build_baseline_program():
    """Same I/O signature, near-zero work: used to subtract dispatch cost."""
    import concourse.bacc as bacc
    import concourse.mybir as mybir
    import concourse.tile as tile

    f32 = mybir.dt.float32
    nc = bacc.Bacc(None, target_bir_lowering=False)
    dec_h = nc.declare_dram_parameter("dec", [TQ, D], f32, isOutput=False)
    nc.declare_dram_parameter("enc", [TK, D], f32, isOutput=False)
    nc.declare_dram_parameter("wa", [D, D], f32, isOutput=False)
    ctx_h = nc.declare_dram_parameter("ctx_out", [TQ, D], f32, isOutput=True)
    aln_h = nc.declare_dram_parameter("aln_out", [TQ, TK], f32, isOutput=True)
    with tile.TileContext(nc) as tc:
        with tc.tile_pool(name="p", bufs=1) as pool:
            t = pool.tile([P, P], f32)
            nc.sync.dma_start(out=t[:], in_=dec_h[:P, :P])
            nc.sync.dma_start(out=ctx_h[:P, :P], in_=t[:])
            nc.sync.dma_start(out=aln_h[:P, :P], in_=t[:])
    nc.finalize()
    return nc


def build_timing_program(R):
    """R sequential repetitions of the kernel body; outputs go to internal
    DRAM (stores still exercised) and only a 128x128 probe is external, so
    per-call host readback is tiny. Slope over R isolates device exec time."""
    import concourse.bacc as bacc
    import concourse.mybir as mybir
    import concourse.tile as tile

    f32 = mybir.dt.float32
    nc = bacc.Bacc(None, target_bir_lowering=False)
    dec_h = nc.declare_dram_parameter("dec", [TQ, D], f32, isOutput=False)
    enc_h = nc.declare_dram_parameter("enc", [TK, D], f32, isOutput=False)
    wa_h = nc.declare_dram_parameter("wa", [D, D], f32, isOutput=False)
    probe = nc.declare_dram_parameter("probe", [P, P], f32, isOutput=True)
    ctx_i = nc.dram_tensor("ctx_i", [TQ, D], f32)
    aln_i = nc.dram_tensor("aln_i", [TQ, TK], f32)
    with tile.TileContext(nc) as tc:
        for _ in range(R):
            _emit_core(nc, tc, dec_h, enc_h, wa_h, ctx_i, aln_i)
        with tc.tile_pool(name="probe_pool", bufs=1) as pp:
            t = pp.tile([P, P], f32)
            nc.sync.dma_start(out=t[:], in_=ctx_i[:P, :P])
            nc.sync.dma_start(out=probe[:], in_=t[:])
    nc.finalize()
    return nc


def time_kernel_slope(inputs, r_lo=1, r_hi=9, reps=15):
    """Device exec time per kernel iteration via (t_hi - t_lo)/(r_hi - r_lo)."""
    import time as _time

    import jax

    dec = np.ascontiguousarray(np.asarray(inputs["decoder_output"], np.float32))
    enc = np.ascontiguousarray(np.asarray(inputs["encoder_output"], np.float32))
    wa = np.ascontiguousarray(np.asarray(Wa_arr := np.asarray(inputs["Wa"], np.float32)))
    in_maps = [{"dec": dec[b], "enc": enc[b], "wa": wa} for b in range(B)]

    def run(R):
        nc = build_timing_program(R)
        fn, in_names, out_names, zero_outs, mesh = _jit_spmd(nc)
        args = _stage_inputs(in_maps, in_names, zero_outs, mesh)
        out = fn(*args)
        jax.block_until_ready(out)
        times = []
        for _ in range(reps):
            t0 = _time.perf_counter()
            out = fn(*args)
            jax.block_until_ready(out)
            times.append(_time.perf_counter() - t0)
        times = np.array(times)
        print(f"  R={R}: median={np.median(times)*1e3:.3f}ms "
              f"min={times.min()*1e3:.3f}ms p25={np.percentile(times,25)*1e3:.3f}ms")
        return times

    t_lo = run(r_lo)
    t_hi = run(r_hi)
    per_iter_med = (np.median(t_hi) - np.median(t_lo)) / (r_hi - r_lo)
    per_iter_min = (t_hi.min() - t_lo.min()) / (r_hi - r_lo)
    print(f"  slope median={per_iter_med*1e6:.1f}us min={per_iter_min*1e6:.1f}us")
    return per_iter_med * 1e9


def time_kernel(inputs, reps=10):
    """Median wall time of the real NEFF minus the empty-NEFF baseline, ns."""
    import time as _time

    import jax

    dec = np.ascontiguousarray(np.asarray(inputs["decoder_output"], np.float32))
    enc = np.ascontiguousarray(np.asarray(inputs["encoder_output"], np.float32))
    wa = np.ascontiguousarray(np.asarray(inputs["Wa"], np.float32))
    in_maps = [{"dec": dec[b], "enc": enc[b], "wa": wa} for b in range(B)]

    def run(nc_builder):
        nc = nc_builder()
        fn, in_names, out_names, zero_outs, mesh = _jit_spmd(nc)
        args = _stage_inputs(in_maps, in_names, zero_outs, mesh)
        out = fn(*args)  # compile + first run
        jax.block_until_ready(out)
        times = []
        for _ in range(reps):
            t0 = _time.perf_counter()
            out = fn(*args)
            jax.block_until_ready(out)
            times.append(_time.perf_counter() - t0)
        return np.median(times), np.min(times), out

    t_base, t_base_min, _ = run(build_baseline_program)
    t_real, t_real_min, out = run(build_program)
    print(f"  baseline median={t_base*1e3:.3f}ms min={t_base_min*1e3:.3f}ms")
    print(f"  real     median={t_real*1e3:.3f}ms min={t_real_min*1e3:.3f}ms")
    return (t_real - t_base) * 1e9


def kernel(decoder_output, encoder_output, Wa, ba):
    # ba is mathematically irrelevant: it shifts each score row by a
    # per-q constant (dec[q,:] @ ba), and softmax is shift-invariant.
    from concourse.bass_utils import run_bass_kernel_spmd

    nc = build_program()
    dec = np.ascontiguousarray(np.asarray(decoder_output, dtype=np.float32))
    enc = np.ascontiguousarray(np.asarray(encoder_output, dtype=np.float32))
    wa = np.ascontiguousarray(np.asarray(Wa, dtype=np.float32))
    in_maps = [
        {"dec": dec[b], "enc": enc[b], "wa": wa} for b in range(B)
    ]
    res = run_bass_kernel_spmd(nc, in_maps, core_ids=list(range(N_CORES)))
    _LAST_RESULT["res"] = res
    context = np.stack([res.results[b]["ctx_out"] for b in range(B)])
    alignment = np.stack([res.results[b]["aln_out"] for b in range(B)])
    return context, alignment
